# revision 5
# baseline (speedup 1.0000x reference)
"""RNNT decoder kernel for TRN2 — 8-core SPMD, T-sharded joint, replicated LSTM.

Layouts (all "transposed": feature dim on partitions):
  hT / cT      [128, (kc4, b4)]            kc = hidden//128
  gates        [128, (mc16, b4)]           gate blocks reordered (i, f, o, g~)
  X0/X1        [128, (u64, mc16, b4)]      precomputed input projections + bias
  H0T/H1T bf16 [128, (kc4, u64, b4)]       rhs for recurrent + batched matmuls
  eysT bf16    [128, (ec4, u64, b4)]
  hencT f32    [128, (jc4, b4, t32)]
  hdecJT f32   [128, (jc4, u64, b4)]
  zT bf16      [128, (jc4, u8, b4, t32)]   per u-block
  out psum     [128o, (u8, b4, t32)]       DMA'd straight to DRAM
"""

import numpy as np
import ml_dtypes

import concourse.bass as bass
import concourse.mybir as mybir
import concourse.tile as tile
from concourse import bacc
from concourse import bass_utils
from concourse.masks import make_identity

B, T, U, E, H, J, OD, G = 4, 256, 64, 512, 512, 512, 1024, 2048
NCORES = 8
TLOC = T // NCORES          # 32
UBLK = 8
NBLK = U // UBLK            # 8
F32 = mybir.dt.float32
BF16 = mybir.dt.bfloat16
I32 = mybir.dt.int32
AF = mybir.ActivationFunctionType
BF = ml_dtypes.bfloat16

_CACHE = {}


def _lstm_layer(nc, P, WK, PS, X, whhT, HT, ctag):
    """Emit one LSTM layer: 64 recurrent steps.
    X: [128, (u, mc, b)] f32 input projections incl. bias.
    whhT: [128, (kc4, j2048)] bf16 stationary recurrent weights (gate-permuted).
    HT: [128, (kc4, u64, b4)] bf16 output hidden states (also the rhs source).
    """
    c = P.tile([128, 16], F32, tag=ctag)
    nc.vector.memset(c[:], 0.0)
    HTv = HT[:].rearrange("p (kc u b) -> p kc u b", kc=4, u=U)
    for u in range(U):
        Xu = X[:, u * 64:(u + 1) * 64]
        if u == 0:
            g_if = Xu[:, 0:32]
            g_og = Xu[:, 32:64]
        else:
            ps_if = PS.tile([128, 32], F32, tag="g")
            ps_og = PS.tile([128, 32], F32, tag="g")
            for half, ps in ((0, ps_if), (1, ps_og)):
                for m in range(8):
                    mc = half * 8 + m
                    for kc in range(4):
                        nc.tensor.matmul(
                            ps[:, m * 4:(m + 1) * 4],
                            lhsT=whhT[:, kc * G + mc * 128: kc * G + mc * 128 + 128],
                            rhs=HT[:, kc * U * 4 + (u - 1) * 4: kc * U * 4 + (u - 1) * 4 + 4],
                            start=(kc == 0), stop=(kc == 3),
                        )
            g_sb = WK.tile([128, 64], F32, tag="g_sb")
            nc.vector.tensor_add(g_sb[:, 0:32], ps_if[:], Xu[:, 0:32])
            nc.vector.tensor_add(g_sb[:, 32:64], ps_og[:], Xu[:, 32:64])
            g_if = g_sb[:, 0:32]
            g_og = g_sb[:, 32:64]
        # gate order after permutation: i (0:16), f (16:32), o (32:48), g~ (48:64)
        s_if = WK.tile([128, 32], F32, tag="s_if")
        nc.scalar.activation(s_if[:], g_if, AF.Sigmoid)
        t_g = WK.tile([128, 16], F32, tag="t_g")
        nc.scalar.activation(t_g[:], g_og[:, 16:32], AF.Tanh)
        s_o = WK.tile([128, 16], F32, tag="s_o")
        nc.scalar.activation(s_o[:], g_og[:, 0:16], AF.Sigmoid)
        if u == 0:
            nc.vector.tensor_mul(c[:], s_if[:, 0:16], t_g[:])
        else:
            t1 = WK.tile([128, 16], F32, tag="t1")
            nc.vector.tensor_mul(t1[:], s_if[:, 16:32], c[:])
            t2 = WK.tile([128, 16], F32, tag="t2")
            nc.vector.tensor_mul(t2[:], s_if[:, 0:16], t_g[:])
            nc.vector.tensor_add(c[:], t1[:], t2[:])
        t_c = WK.tile([128, 16], F32, tag="t_c")
        nc.scalar.activation(t_c[:], c[:], AF.Tanh)
        nc.vector.tensor_mul(
            HTv[:, :, u, :],
            s_o[:].rearrange("p (kc b) -> p kc b", kc=4),
            t_c[:].rearrange("p (kc b) -> p kc b", kc=4),
        )


def _batched_proj(nc, P, PS, wT, rhs_all, bT, Xout):
    """X = (rhs.T @ w).T + b : 16 mc-chunks, psum (u, b), evict strided to (u, mc, b)."""
    Xv = Xout[:].rearrange("p (u mc b) -> p u mc b", u=U, mc=16)
    for mc in range(16):
        ps = PS.tile([128, 256], F32, tag="x")
        for kc in range(4):
            nc.tensor.matmul(
                ps[:],
                lhsT=wT[:, kc * G + mc * 128: kc * G + mc * 128 + 128],
                rhs=rhs_all[:, kc * 256:(kc + 1) * 256],
                start=(kc == 0), stop=(kc == 3),
            )
        nc.vector.tensor_scalar_add(
            Xv[:, :, mc, :],
            ps[:].rearrange("p (u b) -> p u b", u=U),
            bT[:, mc:mc + 1],
        )


def _build():
    nc = bacc.Bacc("TRN2", target_bir_lowering=False, debug=False,
                   enable_asserts=False, num_devices=NCORES)
    hs = nc.dram_tensor("hs", [B, TLOC, E], BF16, kind="ExternalInput").ap()
    emb = nc.dram_tensor("emb", [1024, E], BF16, kind="ExternalInput").ap()
    idx = nc.dram_tensor("idx", [B * U], I32, kind="ExternalInput").ap()
    whh0 = nc.dram_tensor("whh0", [H, G], BF16, kind="ExternalInput").ap()
    wih0 = nc.dram_tensor("wih0", [E, G], BF16, kind="ExternalInput").ap()
    whh1 = nc.dram_tensor("whh1", [H, G], BF16, kind="ExternalInput").ap()
    wih1 = nc.dram_tensor("wih1", [H, G], BF16, kind="ExternalInput").ap()
    wenc = nc.dram_tensor("wenc", [E, J], BF16, kind="ExternalInput").ap()
    wdec = nc.dram_tensor("wdec", [H, J], BF16, kind="ExternalInput").ap()
    wout = nc.dram_tensor("wout", [J, OD], BF16, kind="ExternalInput").ap()
    b0 = nc.dram_tensor("b0", [G], F32, kind="ExternalInput").ap()
    b1 = nc.dram_tensor("b1", [G], F32, kind="ExternalInput").ap()
    benc = nc.dram_tensor("benc", [J], F32, kind="ExternalInput").ap()
    bout = nc.dram_tensor("bout", [OD], F32, kind="ExternalInput").ap()
    # device-native order: [oc, ub, hf, p, u, b, t]; host un-permutes.
    # Partition dim maps to a 2KB-contiguous DRAM row -> each output DMA
    # is one fully contiguous 256KB block (vs 4B-element transpose DMA).
    yout = nc.dram_tensor("out", [8, NBLK, 2, 128, UBLK // 2, B, TLOC], F32,
                          kind="ExternalOutput").ap()

    from contextlib import ExitStack
    with tile.TileContext(nc) as tc, ExitStack() as ctx:
        P = ctx.enter_context(tc.tile_pool(name="persist", bufs=1))
        WK = ctx.enter_context(tc.tile_pool(name="work", bufs=3))
        DBL = ctx.enter_context(tc.tile_pool(name="dbl", bufs=2))
        PS = ctx.enter_context(tc.tile_pool(name="ps", bufs=2, space="PSUM"))

        # ---- weight loads (pre-transposed on host: [K, M*] contiguous) ----
        whh0T = P.tile([128, 4 * G], BF16, tag="whh0T")
        nc.sync.dma_start(whh0T[:].rearrange("p (kc j) -> p kc j", kc=4),
                          whh0.rearrange("(kc p) j -> p kc j", p=128))
        whh1T = P.tile([128, 4 * G], BF16, tag="whh1T")
        nc.sync.dma_start(whh1T[:].rearrange("p (kc j) -> p kc j", kc=4),
                          whh1.rearrange("(kc p) j -> p kc j", p=128))
        wih0T = P.tile([128, 4 * G], BF16, tag="wih")
        nc.sync.dma_start(wih0T[:].rearrange("p (kc j) -> p kc j", kc=4),
                          wih0.rearrange("(kc p) j -> p kc j", p=128))
        wencT = P.tile([128, 4 * J], BF16, tag="wencT")
        nc.sync.dma_start(wencT[:].rearrange("p (kc j) -> p kc j", kc=4),
                          wenc.rearrange("(kc p) j -> p kc j", p=128))
        wdecT = P.tile([128, 4 * J], BF16, tag="wdecT")
        nc.sync.dma_start(wdecT[:].rearrange("p (kc j) -> p kc j", kc=4),
                          wdec.rearrange("(kc p) j -> p kc j", p=128))
        woutT = P.tile([128, 4 * OD], BF16, tag="woutT")
        nc.sync.dma_start(woutT[:].rearrange("p (kc j) -> p kc j", kc=4),
                          wout.rearrange("(kc p) j -> p kc j", p=128))

        # ---- biases ----
        b0T = P.tile([128, 16], F32, tag="b0T")
        nc.sync.dma_start(b0T[:], b0.rearrange("(mc p) -> p mc", p=128))
        b1T = P.tile([128, 16], F32, tag="b1T")
        nc.sync.dma_start(b1T[:], b1.rearrange("(mc p) -> p mc", p=128))
        bencT = P.tile([128, 4], F32, tag="bencT")
        nc.sync.dma_start(bencT[:], benc.rearrange("(jc p) -> p jc", p=128))
        boutT = P.tile([128, 8], F32, tag="boutT")
        nc.sync.dma_start(boutT[:], bout.rearrange("(oc p) -> p oc", p=128))

        # ---- embedding gather + transpose -> eysT [128, (ec, u, b)] ----
        idx_sb = P.tile([128, 2], I32, tag="idx")
        for r in range(2):
            nc.sync.dma_start(idx_sb[:, r:r + 1], idx[r * 128:(r + 1) * 128].unsqueeze(1))
        ident = P.tile([128, 128], BF16, tag="ident")
        make_identity(nc, ident[:])
        eysT = P.tile([128, 4 * 256], BF16, tag="eysT")
        for r in range(2):
            eys_sb = P.tile([128, E], BF16, tag=f"eys{r}")
            nc.gpsimd.indirect_dma_start(
                out=eys_sb[:], out_offset=None, in_=emb,
                in_offset=bass.IndirectOffsetOnAxis(ap=idx_sb[:, r:r + 1], axis=0))
            for ec in range(4):
                pst = PS.tile([128, 128], BF16, tag="tpbig")
                nc.tensor.transpose(out=pst[:], in_=eys_sb[:, ec * 128:(ec + 1) * 128],
                                    identity=ident[:])
                nc.vector.tensor_copy(eysT[:, ec * 256 + r * 128: ec * 256 + r * 128 + 128],
                                      pst[:])

        # ---- hs slice -> hsT [128, (ec, b, t)] ----
        hs_sb = P.tile([128, E], BF16, tag="hs_sb")
        for b in range(B):
            nc.sync.dma_start(hs_sb[b * TLOC:(b + 1) * TLOC, :], hs[b])
        hsT = P.tile([128, 4 * 128], BF16, tag="hsT")
        for ec in range(4):
            pst = PS.tile([128, 128], BF16, tag="tpbig")
            nc.tensor.transpose(out=pst[:], in_=hs_sb[:, ec * 128:(ec + 1) * 128],
                                identity=ident[:])
            nc.vector.tensor_copy(hsT[:, ec * 128:(ec + 1) * 128], pst[:])

        # ---- henc -> hencT [128, (jc, b, t)] f32 ----
        hencT = P.tile([128, 4 * 128], F32, tag="hencT")
        for jc in range(4):
            ps = PS.tile([128, 128], F32, tag="x")
            for kc in range(4):
                nc.tensor.matmul(
                    ps[:], lhsT=wencT[:, kc * J + jc * 128: kc * J + jc * 128 + 128],
                    rhs=hsT[:, kc * 128:(kc + 1) * 128],
                    start=(kc == 0), stop=(kc == 3))
            nc.vector.tensor_scalar_add(hencT[:, jc * 128:(jc + 1) * 128], ps[:],
                                        bencT[:, jc:jc + 1])

        # ---- X0 = eys @ Wih0.T + b0 ----
        X0 = P.tile([128, U * 64], F32, tag="X")
        _batched_proj(nc, P, PS, wih0T, eysT, b0T, X0)

        # ---- layer 0 ----
        H0T = P.tile([128, 4 * U * B], BF16, tag="H0T")
        _lstm_layer(nc, P, WK, PS, X0, whh0T, H0T, "c0")

        # ---- X1 = H0 @ Wih1.T + b1 (reuses wih + X slots) ----
        wih1T = P.tile([128, 4 * G], BF16, tag="wih")
        nc.sync.dma_start(wih1T[:].rearrange("p (kc j) -> p kc j", kc=4),
                          wih1.rearrange("(kc p) j -> p kc j", p=128))
        X1 = P.tile([128, U * 64], F32, tag="X")
        _batched_proj(nc, P, PS, wih1T, H0T, b1T, X1)

        # ---- layer 1 ----
        H1T = P.tile([128, 4 * U * B], BF16, tag="H1T")
        _lstm_layer(nc, P, WK, PS, X1, whh1T, H1T, "c1")

        # ---- hdecJ = h_dec @ W_dec.T -> hdecJT [128, (jc, u, b)] f32 ----
        hdecJT = P.tile([128, 4 * 256], F32, tag="hdecJT")
        for jc in range(4):
            ps = PS.tile([128, 256], F32, tag="x")
            for kc in range(4):
                nc.tensor.matmul(
                    ps[:], lhsT=wdecT[:, kc * J + jc * 128: kc * J + jc * 128 + 128],
                    rhs=H1T[:, kc * 256:(kc + 1) * 256],
                    start=(kc == 0), stop=(kc == 3))
            nc.vector.tensor_copy(hdecJT[:, jc * 256:(jc + 1) * 256], ps[:])

        # ---- joint, per u-block ----
        for ub in range(NBLK):
            zT = DBL.tile([128, 4 * UBLK * B * TLOC], BF16, tag="zT")
            for jc in range(4):
                zin = DBL.tile([128, UBLK * B * TLOC], F32, tag="zin")
                henc_bc = (hencT[:, jc * 128:(jc + 1) * 128]
                           .rearrange("p (b t) -> p b t", b=B)
                           .unsqueeze(1).to_broadcast([128, UBLK, B, TLOC]))
                hdec_bc = (hdecJT[:, jc * 256 + ub * UBLK * B: jc * 256 + (ub + 1) * UBLK * B]
                           .rearrange("p (u b) -> p u b", u=UBLK)
                           .unsqueeze(3).to_broadcast([128, UBLK, B, TLOC]))
                nc.vector.tensor_add(
                    zin[:].rearrange("p (u b t) -> p u b t", u=UBLK, b=B),
                    henc_bc, hdec_bc)
                nc.scalar.activation(zT[:, jc * 1024:(jc + 1) * 1024], zin[:], AF.Tanh)
            for oc in range(8):
                for hf in range(2):
                    ps = PS.tile([128, 512], F32, tag="tpbig")
                    for jc in range(4):
                        nc.tensor.matmul(
                            ps[:],
                            lhsT=woutT[:, jc * OD + oc * 128: jc * OD + oc * 128 + 128],
                            rhs=zT[:, jc * 1024 + hf * 512: jc * 1024 + hf * 512 + 512],
                            start=(jc == 0), stop=(jc == 3))
                    zout = DBL.tile([128, 512], F32, tag="zout")
                    nc.vector.tensor_scalar_add(zout[:], ps[:], boutT[:, oc:oc + 1])
                    nc.sync.dma_start(
                        yout[oc, ub, hf],
                        zout[:].rearrange("p (u b t) -> p u b t", u=UBLK // 2, b=B))
    nc.compile()
    return nc


def _get_nc():
    if "nc" not in _CACHE:
        _CACHE["nc"] = _build()
    return _CACHE["nc"]


# torch gate order (i, f, g, o) -> device order (i, f, o, g~)
_PERM = np.concatenate([np.arange(0, 512), np.arange(512, 1024),
                        np.arange(1536, 2048), np.arange(1024, 1536)])


def _prep_w(w):
    """[2048, 512] f32 -> [512, 2048] bf16, gate-permuted."""
    return np.ascontiguousarray(np.asarray(w, np.float32)[_PERM].T).astype(BF)


def kernel(**inputs):
    nc = _get_nc()
    hs_pad = np.asarray(inputs["hs_pad"], np.float32)
    ys_pad = np.asarray(inputs["ys_pad"])
    embed = np.asarray(inputs["embed"], np.float32)

    ys_in = np.concatenate([np.zeros((B, 1), ys_pad.dtype), ys_pad], axis=1)
    idx = np.ascontiguousarray(ys_in.T).reshape(-1).astype(np.int32)  # u-major

    common = {
        "emb": embed.astype(BF),
        "idx": idx,
        "whh0": _prep_w(inputs["W_hh0"]),
        "wih0": _prep_w(inputs["W_ih0"]),
        "whh1": _prep_w(inputs["W_hh1"]),
        "wih1": _prep_w(inputs["W_ih1"]),
        "wenc": np.ascontiguousarray(np.asarray(inputs["W_enc"], np.float32).T).astype(BF),
        "wdec": np.ascontiguousarray(np.asarray(inputs["W_dec"], np.float32).T).astype(BF),
        "wout": np.ascontiguousarray(np.asarray(inputs["W_out"], np.float32).T).astype(BF),
        "b0": (np.asarray(inputs["b_ih0"], np.float32)
               + np.asarray(inputs["b_hh0"], np.float32))[_PERM].copy(),
        "b1": (np.asarray(inputs["b_ih1"], np.float32)
               + np.asarray(inputs["b_hh1"], np.float32))[_PERM].copy(),
        "benc": np.asarray(inputs["b_enc"], np.float32),
        "bout": np.asarray(inputs["b_out"], np.float32),
    }
    in_maps = []
    for c in range(NCORES):
        m = dict(common)
        m["hs"] = np.ascontiguousarray(
            hs_pad[:, c * TLOC:(c + 1) * TLOC, :]).astype(BF)
        in_maps.append(m)

    _CACHE["in_maps"] = in_maps
    res = bass_utils.run_bass_kernel_spmd(nc, in_maps, core_ids=list(range(NCORES)))
    outs = []
    for r in res.results:
        o = np.asarray(r["out"]).reshape(8, NBLK, 2, 128, UBLK // 2, B, TLOC)
        outs.append(np.transpose(o, (5, 6, 1, 2, 4, 0, 3)).reshape(B, TLOC, U, OD))
    return np.concatenate(outs, axis=1).astype(np.float32)

